# revision 9
# baseline (speedup 1.0000x reference)
"""Trainium2 Bass kernel for ContinuousValueEncoderWithSpecialTokenEmbeddings.

out[b,s,:] = leaky(merged) @ W2 + b2 where merged is emb[mapped] for special
tokens (v<=0) and leaky(v*w1+b1) for continuous tokens.

Strategy (per core, data-parallel over batch, 1 row of 4096 tokens each):
  - one-hot over the 6 special rows via is_equal against {-1..-5, 0};
    relu(v) as the masked continuous value row
  - pre-activation built by 3 accumulating fp32r matmuls into PSUM:
      embPre2.T @ onehot + b1 x ones + w1 x relu(v)
    where embPre2 = min(emb, 100*emb) - b1 is the dleaky-preimage of
    leaky(emb) (minus b1, which the ones term adds back unconditionally)
  - a = dleaky(pre) = LeakyReLU_{1e-4}(pre)  [= leaky(leaky(x)) on both paths]
  - out = a.T @ W2 (+ b2 via K=1 accumulate matmul), evictions are plain
    copies split across ScalarE/VectorE, stores grouped 2 subtiles per DMA
"""

import sys

for _p in ("/opt/trn_rl_repo",):
    if _p not in sys.path:
        sys.path.insert(0, _p)

import numpy as np

import concourse.bacc as bacc
import concourse.mybir as mybir
import concourse.tile as tile
from concourse.bass_utils import run_bass_kernel_spmd

B, S = 8, 4096
HID, HS = 128, 768
N_CORES = 8
NTOK = S            # tokens per core (shard along batch)
T = 512             # tokens per pre-activation tile (one PSUM bank)
M = 128             # tokens per output matmul subtile
G = 2               # output subtiles per store DMA
SLOPE = 0.01
F32 = mybir.dt.float32
F32R = mybir.dt.float32r

# packed small-param layout: w1[0:128] b1[128:256] cvals[256:262] b2[262:1030]
PK = 1030


def _build():
    nc = bacc.Bacc()
    v_d = nc.dram_tensor("v", [NTOK], F32, kind="ExternalInput")
    pk_d = nc.dram_tensor("pk", [1, PK], F32, kind="ExternalInput")
    W2_d = nc.dram_tensor("W2", [HID, HS], F32, kind="ExternalInput")
    emb_d = nc.dram_tensor("emb", [6, HID], F32, kind="ExternalInput")
    out_d = nc.dram_tensor("out", [NTOK, HS], F32, kind="ExternalOutput")

    NTT = NTOK // T
    NSUB = T // M

    with tile.TileContext(nc) as tc:
        with (
            tc.tile_pool(name="const", bufs=1) as cpool,
            tc.tile_pool(name="work", bufs=3) as wpool,
            tc.tile_pool(name="outp", bufs=4) as opool,
            tc.tile_pool(name="psa", bufs=2, space="PSUM") as pspool,
            tc.tile_pool(name="pso", bufs=3, space="PSUM") as popool,
        ):
            pk = cpool.tile([1, PK], F32)
            cvals = cpool.tile([6, 1], F32)
            b1rep = cpool.tile([6, HID], F32)
            embP = cpool.tile([6, HID], F32)
            tmp6 = cpool.tile([6, HID], F32)
            W2t = cpool.tile([HID, HS], F32)
            vrep6 = cpool.tile([6, NTOK], F32)
            v1 = cpool.tile([1, NTOK], F32)
            # fp32r operand tiles (fp32r matmul inputs must be pre-rounded)
            w1r = cpool.tile([1, HID], F32R)
            b1r = cpool.tile([1, HID], F32R)
            b2r = cpool.tile([1, HS], F32R)
            embPr = cpool.tile([6, HID], F32R)
            W2r = cpool.tile([HID, HS], F32R)
            ones1f = cpool.tile([1, T], F32)
            ones1 = cpool.tile([1, T], F32R)
            teq = cpool.tile([6, NTOK], F32R)
            tmax = cpool.tile([1, NTOK], F32R)

            # input DMAs on the scalar HWDGE ring, critical-path first;
            # output stores go on the sync ring concurrently
            nc.scalar.dma_start(out=pk[:], in_=pk_d[:])
            nc.scalar.dma_start(out=cvals[:], in_=pk_d[0, 256:262][:, None])
            nc.scalar.dma_start(out=embP[:], in_=emb_d[:])
            nc.scalar.dma_start(out=b1rep[:],
                                in_=pk_d[0:1, 128:256].broadcast_to([6, HID]))
            nc.scalar.dma_start(out=v1[:], in_=v_d[None, :])
            nc.scalar.dma_start(out=vrep6[:],
                                in_=v_d[None, :].broadcast_to([6, NTOK]))
            nc.scalar.dma_start(out=W2t[:], in_=W2_d[:])

            nc.vector.memset(ones1f[:], 1.0)
            nc.vector.tensor_copy(ones1[:], ones1f[:])
            nc.vector.tensor_copy(w1r[:], pk[:, 0:128])
            nc.vector.tensor_copy(b1r[:], pk[:, 128:256])
            nc.vector.tensor_copy(b2r[:], pk[:, 262:1030])
            nc.vector.tensor_copy(W2r[:], W2t[:])
            # embPre2 = min(emb, 100*emb) - b1
            nc.scalar.mul(tmp6[:], embP[:], 100.0)
            nc.vector.tensor_tensor(embP[:], embP[:], tmp6[:], mybir.AluOpType.min)
            nc.vector.tensor_tensor(embPr[:], embP[:], b1rep[:],
                                    mybir.AluOpType.subtract)

            for tt in range(NTT):
                ts = slice(tt * T, (tt + 1) * T)
                nc.vector.tensor_scalar(teq[:, ts], vrep6[:, ts], cvals[:], None,
                                        mybir.AluOpType.is_equal)
                nc.vector.tensor_scalar(tmax[:, ts], v1[:, ts], 0.0, None,
                                        mybir.AluOpType.max)
                psa = pspool.tile([HID, T], F32)
                nc.tensor.matmul(psa[:], embPr[:], teq[:, ts],
                                 start=True, stop=False)
                nc.tensor.matmul(psa[:], b1r[:], ones1[:],
                                 start=False, stop=False)
                nc.tensor.matmul(psa[:], w1r[:], tmax[:, ts],
                                 start=False, stop=True)
                # a = dleaky(psa) = max(psa, 1e-4*psa), rounded to fp32r
                a_tmp = wpool.tile([HID, T], F32, tag="a_tmp")
                a_sb = wpool.tile([HID, T], F32R, tag="a_sb")
                nc.scalar.mul(a_tmp[:], psa[:], SLOPE * SLOPE)
                nc.vector.tensor_tensor(a_sb[:], psa[:], a_tmp[:],
                                        mybir.AluOpType.max)
                for m in range(NSUB):
                    ms = slice(m * M, (m + 1) * M)
                    po0 = popool.tile([M, 512], F32, tag="po0")
                    po1 = popool.tile([M, 256], F32, tag="po1")
                    nc.tensor.matmul(po0[:], a_sb[:, ms], W2r[:, 0:512],
                                     start=True, stop=False)
                    nc.tensor.matmul(po0[:], ones1[:, 0:M], b2r[:, 0:512],
                                     start=False, stop=True)
                    nc.tensor.matmul(po1[:], a_sb[:, ms], W2r[:, 512:768],
                                     start=True, stop=False)
                    nc.tensor.matmul(po1[:], ones1[:, 0:M], b2r[:, 512:768],
                                     start=False, stop=True)
                    g = m % G
                    if g == 0:
                        osb = opool.tile([M, G * HS], F32)
                    nc.vector.tensor_copy(osb[:, g * HS: g * HS + 512], po0[:])
                    nc.vector.tensor_copy(osb[:, g * HS + 512: (g + 1) * HS],
                                          po1[:])
                    if g == G - 1:
                        row0 = tt * T + (m - G + 1) * M
                        dview = out_d[row0: row0 + G * M, :].rearrange(
                            "(g p) h -> p g h", g=G)
                        sview = osb[:].rearrange("p (g h) -> p g h", g=G)
                        nc.sync.dma_start(out=dview, in_=sview)

    nc.compile()
    return nc


_CACHE = {}


def _get_nc():
    if "nc" not in _CACHE:
        _CACHE["nc"] = _build()
    return _CACHE["nc"]


def _run(inputs, trace=False, trace_kwargs=None):
    nc = _get_nc()
    w1 = np.ascontiguousarray(inputs["w1"], np.float32).reshape(HID)
    b1 = np.ascontiguousarray(inputs["b1"], np.float32).reshape(HID)
    b2 = np.ascontiguousarray(inputs["b2"], np.float32).reshape(HS)
    cvals = np.array([-1.0, -2.0, -3.0, -4.0, -5.0, 0.0], np.float32)
    pk = np.concatenate([w1, b1, cvals, b2]).reshape(1, PK)
    base = {
        "pk": pk,
        "W2": np.ascontiguousarray(inputs["W2"], np.float32),
        "emb": np.ascontiguousarray(inputs["emb"], np.float32),
    }
    iv = np.ascontiguousarray(inputs["input_value"], np.float32)
    in_maps = [dict(base, v=iv[i]) for i in range(N_CORES)]
    kw = {}
    if trace:
        kw["trace"] = True
        if trace_kwargs:
            kw["trace_kwargs"] = trace_kwargs
    res = run_bass_kernel_spmd(nc, in_maps, list(range(N_CORES)), **kw)
    out = np.stack([res.results[i]["out"] for i in range(N_CORES)], axis=0)
    return out.astype(np.float32, copy=False), res


def kernel(**inputs):
    out, _ = _run(inputs)
    return out
